# revision 24
# baseline (speedup 1.0000x reference)
"""Trainium2 Bass kernel for nn_AggFeatureModel (segment_reduce).

Computes, per batch row b (B=4096, T=2048):
  - seq_len, sum/mean/std of amount over the full T axis
  - per-category (mcc: C=100, tr_type: C=50) count/mean/std of amount
  - distinct-category counts
Output: [B, 456].

Sharding: pure data parallel, B split across 8 NeuronCores (512 rows each).

Algorithm (per 128-row tile, per categorical stream):
 1. Partition each row's 2048 elements into 13 contiguous category-range
    groups via fused custom DVE ops (range mask + prefix-scan rank + dest,
    one op per group, chained through a fill tensor; last op emits int16).
 2. gpsimd local_scatter rearranges (cat, val) into fixed per-group windows
    of a compact grid (two scatter halves, each < 2048 dst elems).
    Phase emission is software-pipelined one tile ahead so scatters overlap
    the previous tile's accumulation.
 3. Per category, two fused scalar_tensor_tensor+accum ops over only that
    category's ~240-wide group window (instead of 2048):
      u  = sum((cat==c) * (val + 32768))  -> cnt = round(u/2^15), s = u - 2^15*cnt
      ss = sum((cat==c) * val^2)
 4. Postprocess cnt/mean/std + row stats + distinct counts.
"""

import sys

sys.path.insert(0, "/opt/trn_rl_repo")

from contextlib import ExitStack

import numpy as np

import concourse.tile as tile
from concourse import bacc, mybir, library_config
from concourse import dve_ops as DO
from concourse.dve_spec import Spec, Src0, Src1, C0, C1, C2, Zero, One, select, scan, lower
from concourse.dve_uop import DveOpSpec, AluOp
from concourse.dve_ops import has_src1
from concourse.bass_utils import run_bass_kernel_spmd

B, T = 4096, 2048
NCORES = 8
RPC = B // NCORES
C_MCC, C_TR = 100, 50
EPS = 1e-9
OUT_COLS = 456
PT = 128
NT = RPC // PT

PACK = 128.0  # cnt/s packing offset (fp16 payload: ulp(128)=0.125)

F32 = mybir.dt.float32
BF16 = mybir.dt.bfloat16
F16 = mybir.dt.float16
I16 = mybir.dt.int16
I32 = mybir.dt.int32
AX = mybir.AxisListType.X
OP = mybir.AluOpType
AF = mybir.ActivationFunctionType

# --- group layout -----------------------------------------------------------
# mcc: cat 1..99, 13 groups of 8 (g = (c-1)>>3); tr: cat 1..49, 13 groups of 4.
MCC_W = [240] * 12 + [110]
TR_W = [240] * 12 + [80]
MCC_NH0 = 6  # groups in scatter half 0
TR_NH0 = 6
MCC_BOUNDS = [(8 * g + 1, 8 * g + 9 if g < 12 else 16384) for g in range(13)]
TR_BOUNDS = [(4 * g + 1, 4 * g + 5 if g < 12 else 16384) for g in range(13)]


def _layout(widths, nhalf0):
    """Return (total, half0_len, half1_len, global_offsets, local_offsets)."""
    goff, loff = [], []
    g0 = 0
    for i, w in enumerate(widths[:nhalf0]):
        goff.append(g0)
        loff.append(g0)
        g0 += w
    h0 = g0
    l1 = 0
    for w in widths[nhalf0:]:
        goff.append(h0 + l1)
        loff.append(l1)
        l1 += w
    return h0 + l1, h0, l1, goff, loff


MCC_TOT, MCC_H0, MCC_H1, MCC_GOFF, MCC_LOFF = _layout(MCC_W, MCC_NH0)
TR_TOT, TR_H0, TR_H1, TR_GOFF, TR_LOFF = _layout(TR_W, TR_NH0)
assert MCC_H0 <= 2046 and MCC_H1 <= 2046 and TR_H0 <= 2046 and TR_H1 <= 2046

# --- custom DVE ops ---------------------------------------------------------
_m = (Src0 >= C0) & (Src0 < C1)
_r = scan(AluOp.ADD, _m)


def _ref_init(in0, in1, s0, s1, imm2):
    m = (in0 >= s0) & (in0 < s1)
    r = np.cumsum(m.reshape(in0.shape[0], -1), axis=-1).reshape(in0.shape)
    return np.where(m, r + imm2, -1.0).astype(np.float32)


def _ref_acc(in0, in1, s0, s1, imm2):
    m = (in0 >= s0) & (in0 < s1)
    r = np.cumsum(m.reshape(in0.shape[0], -1), axis=-1).reshape(in0.shape)
    return np.where(m, r + imm2, in1).astype(np.float32)


def _register_op(name, spec, subdim=False):
    for op in DO.OPS:
        if op.name == name:
            return op
    row = DO._CUSTOM_DVE_ROW_BASE + len(DO.OPS)
    assert row < 0x20
    shas = {}
    for ver in ("v3", "v4"):
        uops = lower(spec, ver=ver)
        s = DveOpSpec(name=name, opcode=row, uops=uops, rd1_en=has_src1(spec))
        shas[ver] = s.sha(ver)
    op = DO.DveOp(name, spec, subdim=subdim, uops_sha=shas)
    DO.OPS.append(op)
    DO._SUB_OPCODE_FOR_NAME[name] = row
    DO.CUSTOM_DVE_SPECS[name] = spec
    return op


GROUP_DEST_INIT = _register_op(
    "GROUP_DEST_INIT", Spec(body=select(_m, _r + C2, Zero - One), reference=_ref_init)
)
GROUP_DEST_ACC = _register_op(
    "GROUP_DEST_ACC", Spec(body=select(_m, _r + C2, Src1), reference=_ref_acc)
)


def _cat_stats_postproc(nc, pool, cnt, s, ss, out_tile, col0, C, epsb):
    """cnt/s/ss [128, C] f32 -> out_tile[:, col0:col0+3C]; returns distinct."""
    tmp = pool.tile([PT, C], F32, tag=f"pp_tmp_{C}")
    rec = pool.tile([PT, C], F32, tag=f"pp_rec_{C}")
    nc.vector.tensor_copy(out_tile[:, col0 : col0 + C], cnt[:])
    nc.vector.tensor_scalar(tmp[:], cnt[:], EPS, None, OP.add)
    nc.vector.reciprocal(rec[:], tmp[:])
    mean = out_tile[:, col0 + C : col0 + 2 * C]
    nc.vector.tensor_tensor(mean, s[:], rec[:], OP.mult)
    nc.vector.tensor_tensor(tmp[:], s[:], mean, OP.mult)
    nc.vector.tensor_tensor(tmp[:], ss[:], tmp[:], OP.subtract)
    nc.vector.tensor_scalar(tmp[:], tmp[:], 0.0, None, OP.max)
    nc.vector.tensor_scalar(rec[:], cnt[:], 1.0, 0.0, OP.subtract, OP.max)
    nc.vector.tensor_scalar(rec[:], rec[:], EPS, None, OP.add)
    nc.vector.reciprocal(rec[:], rec[:])
    nc.vector.tensor_tensor(tmp[:], tmp[:], rec[:], OP.mult)
    nc.scalar.sqrt(out_tile[:, col0 + 2 * C : col0 + 3 * C], tmp[:])
    dist = pool.tile([PT, 1], F32, tag=f"pp_dist_{C}")
    nc.vector.tensor_scalar(
        tmp[:], cnt[:], 0.0, 0.0, OP.is_gt, OP.add, accum_out=dist[:]
    )
    return dist


def _stream(nc, work, scat, cat_bf, val_bf, tag, widths, tot, h0len, h1len,
            goff, loff, bounds, nh0, packb=None):
    """Group-scatter one categorical stream; return (cat_s, in1f, v2_s)."""
    ngrp = len(widths)
    # dest computation: chained custom ops, one per group; two halves;
    # ping-pong between two scratch tiles (all DVE, single-buffered)
    dh = [
        work.tile([PT, T], F16, tag=f"dhA_{tag}", name=f"dhA_{tag}", bufs=1),
        work.tile([PT, T], F16, tag=f"dhB_{tag}", name=f"dhB_{tag}", bufs=1),
    ]
    idxs = []
    for half, grange in ((0, range(0, nh0)), (1, range(nh0, ngrp))):
        dprev = None
        idx = work.tile(
            [PT, T], I16, tag=f"i{half}_{tag}", name=f"i{half}_{tag}"
        )
        glist = list(grange)
        for k, g in enumerate(glist):
            op = GROUP_DEST_INIT if k == 0 else GROUP_DEST_ACC
            kw = {} if k == 0 else {"in1": dprev[:]}
            last = k == len(glist) - 1
            cur = idx if last else dh[k % 2]
            nc.vector._custom_dve(
                op, out=cur[:], in0=cat_bf[:],
                s0=float(bounds[g][0]), s1=float(bounds[g][1]),
                imm2=float(loff[g] - 1), **kw,
            )
            dprev = cur
        idxs.append(idx)
    idx0, idx1 = idxs

    cat_s = scat.tile([PT, tot], BF16, tag=f"cs_{tag}")
    val_s = scat.tile([PT, tot], BF16, tag=f"vs_{tag}")
    nc.gpsimd.local_scatter(
        cat_s[:, 0:h0len], cat_bf[:], idx0[:], channels=PT,
        num_elems=h0len, num_idxs=T,
    )
    nc.gpsimd.local_scatter(
        cat_s[:, h0len:tot], cat_bf[:], idx1[:], channels=PT,
        num_elems=h1len, num_idxs=T,
    )
    nc.gpsimd.local_scatter(
        val_s[:, 0:h0len], val_bf[:], idx0[:], channels=PT,
        num_elems=h0len, num_idxs=T,
    )
    nc.gpsimd.local_scatter(
        val_s[:, h0len:tot], val_bf[:], idx1[:], channels=PT,
        num_elems=h1len, num_idxs=T,
    )

    # payloads: in1f = val_s + PACK (f32, ACT); v2_s = val_s^2 (bf16, ACT)
    in1f = scat.tile([PT, tot], F16, tag=f"p_{tag}", bufs=1)
    nc.scalar.activation(in1f[:], val_s[:], AF.Identity, bias=packb[:])
    v2_s = scat.tile([PT, tot], BF16, tag=f"v2_{tag}", bufs=1)
    nc.scalar.activation(v2_s[:], val_s[:], AF.Square)
    return cat_s, in1f, v2_s


def _build_body(ctx, tc):
    nc = tc.nc
    amount_d = nc.dram_tensor("amount", [RPC, T], F32, kind="ExternalInput")
    mcc_d = nc.dram_tensor("mcc", [RPC, T], I32, kind="ExternalInput")
    tr_d = nc.dram_tensor("tr_type", [RPC, T], I32, kind="ExternalInput")
    seq_d = nc.dram_tensor("seq_lens", [RPC, 1], I32, kind="ExternalInput")
    out_d = nc.dram_tensor("out", [RPC, OUT_COLS], F32, kind="ExternalOutput")

    io_pool = ctx.enter_context(tc.tile_pool(name="io", bufs=1))
    work = ctx.enter_context(tc.tile_pool(name="work", bufs=2))
    scat = ctx.enter_context(tc.tile_pool(name="scat", bufs=2))
    acc_pool = ctx.enter_context(tc.tile_pool(name="acc", bufs=2))

    nc.gpsimd.load_library(library_config.local_scatter)

    packb_box = [None]
    epsb_box = [None]

    def phase_a(it):
        r0 = it * PT
        rows = slice(r0, r0 + PT)

        a = io_pool.tile([PT, T], F32, tag="a")
        nc.sync.dma_start(a[:], amount_d[rows, :])
        mcc_i = io_pool.tile([PT, T], I32, tag="mcc_i")
        nc.sync.dma_start(mcc_i[:], mcc_d[rows, :])
        tr_i = io_pool.tile([PT, T], I32, tag="tr_i")
        nc.sync.dma_start(tr_i[:], tr_d[rows, :])
        seq_i = io_pool.tile([PT, 1], I32, tag="seq_i")
        nc.sync.dma_start(seq_i[:], seq_d[rows, :])

        if packb_box[0] is None:
            packb_box[0] = work.tile([PT, 1], F32, tag="packb", name="packb", bufs=1)
            nc.vector.memset(packb_box[0][:], PACK)
            epsb_box[0] = work.tile([PT, 1], F32, tag="epsb", name="epsb", bufs=1)
            nc.vector.memset(epsb_box[0][:], EPS)
        packb = packb_box[0]
        seq_f = work.tile([PT, 1], F32, tag="seq_f")
        nc.vector.tensor_copy(seq_f[:], seq_i[:])
        mcc_bf = work.tile([PT, T], BF16, tag="mcc_bf")
        nc.scalar.copy(mcc_bf[:], mcc_i[:])
        tr_bf = work.tile([PT, T], BF16, tag="tr_bf")
        nc.scalar.copy(tr_bf[:], tr_i[:])
        val_bf = work.tile([PT, T], BF16, tag="val_bf")
        nc.scalar.copy(val_bf[:], a[:])

        # row stats on the unsorted tile
        ss_row = work.tile([PT, 1], F32, tag="ss_row")
        jact = work.tile([PT, T], BF16, tag="jact", bufs=1)
        nc.scalar.activation(jact[:], a[:], AF.Square, accum_out=ss_row[:])
        s_row = work.tile([PT, 1], F32, tag="s_row")
        nc.vector.reduce_sum(s_row[:], a[:], axis=AX)

        cat_sm, in1f_m, v2_sm = _stream(
            nc, work, scat, mcc_bf, val_bf, "m", MCC_W, MCC_TOT,
            MCC_H0, MCC_H1, MCC_GOFF, MCC_LOFF, MCC_BOUNDS, MCC_NH0, packb,
        )
        cat_st, in1f_t, v2_st = _stream(
            nc, work, scat, tr_bf, val_bf, "t", TR_W, TR_TOT,
            TR_H0, TR_H1, TR_GOFF, TR_LOFF, TR_BOUNDS, TR_NH0, packb,
        )
        return dict(
            rows=rows, seq_f=seq_f, s_row=s_row, ss_row=ss_row,
            cat_sm=cat_sm, in1f_m=in1f_m, v2_sm=v2_sm,
            cat_st=cat_st, in1f_t=in1f_t, v2_st=v2_st,
        )

    def phase_b(st):
        rows = st["rows"]
        seq_f, s_row, ss_row = st["seq_f"], st["s_row"], st["ss_row"]
        cat_sm, in1f_m, v2_sm = st["cat_sm"], st["in1f_m"], st["v2_sm"]
        cat_st, in1f_t, v2_st = st["cat_st"], st["in1f_t"], st["v2_st"]

        out_tile = acc_pool.tile([PT, OUT_COLS], F32, tag="out_tile")

        for tag, cat_s, in1f, v2_s, widths, goff, cpg, C, col0, dcol in (
            ("m", cat_sm, in1f_m, v2_sm, MCC_W, MCC_GOFF, 8, C_MCC, 4, 454),
            ("t", cat_st, in1f_t, v2_st, TR_W, TR_GOFF, 4, C_TR,
             4 + 3 * C_MCC, 455),
        ):
            u_acc = acc_pool.tile([PT, C], F32, tag=f"u_{tag}", name=f"u_{tag}")
            ss_acc = acc_pool.tile([PT, C], F32, tag=f"q_{tag}", name=f"q_{tag}")
            nc.vector.memset(u_acc[:, 0:1], 0.0)
            nc.vector.memset(ss_acc[:, 0:1], 0.0)
            sv0 = work.tile([PT, 240], BF16, tag=f"sv0_{tag}", name=f"sv0_{tag}")
            sv1 = work.tile([PT, 240], F16, tag=f"sv1_{tag}", name=f"sv1_{tag}")
            for c in range(1, C):
                g = (c - 1) // cpg
                w = widths[g]
                win = slice(goff[g], goff[g] + w)
                fc = float(c)
                # u = sum((cat==c) * (val + PACK))  [cnt+s packed]
                nc.vector.scalar_tensor_tensor(
                    sv1[:, 0:w], cat_s[:, win], fc, in1f[:, win],
                    OP.is_equal, OP.mult, accum_out=u_acc[:, c : c + 1],
                )
                # ss = sum((cat==c) * val^2)
                nc.vector.scalar_tensor_tensor(
                    sv0[:, 0:w], cat_s[:, win], fc, v2_s[:, win],
                    OP.is_equal, OP.mult, accum_out=ss_acc[:, c : c + 1],
                )

            # decode cnt/s from u = PACK*cnt + s via int convert, then a
            # +/-1 correction that works whether the convert rounds or
            # truncates
            cnt_f = acc_pool.tile([PT, C], F32, tag=f"cf_{tag}", name=f"cf_{tag}")
            cnt_i = acc_pool.tile([PT, C], I32, tag=f"ci_{tag}", name=f"ci_{tag}")
            s_dec = acc_pool.tile([PT, C], F32, tag=f"sd_{tag}", name=f"sd_{tag}")
            fix = acc_pool.tile([PT, C], F32, tag=f"fx_{tag}", name=f"fx_{tag}")
            nc.vector.tensor_scalar(cnt_f[:], u_acc[:], 1.0 / PACK, None, OP.mult)
            nc.vector.tensor_copy(cnt_i[:], cnt_f[:])
            nc.vector.tensor_copy(cnt_f[:], cnt_i[:])
            nc.vector.tensor_scalar(s_dec[:], cnt_f[:], -PACK, None, OP.mult)
            nc.vector.tensor_tensor(s_dec[:], u_acc[:], s_dec[:], OP.add)
            # if s > PACK/2: cnt += 1, s -= PACK
            nc.vector.tensor_scalar(fix[:], s_dec[:], PACK / 2, None, OP.is_gt)
            nc.vector.tensor_tensor(cnt_f[:], cnt_f[:], fix[:], OP.add)
            nc.vector.tensor_scalar(fix[:], fix[:], -PACK, None, OP.mult)
            nc.vector.tensor_tensor(s_dec[:], s_dec[:], fix[:], OP.add)
            # zero out category 0 columns
            nc.vector.memset(cnt_f[:, 0:1], 0.0)
            nc.vector.memset(s_dec[:, 0:1], 0.0)
            nc.vector.memset(ss_acc[:, 0:1], 0.0)

            dist = _cat_stats_postproc(
                nc, work, cnt_f, s_dec, ss_acc, out_tile, col0, C,
                epsb_box[0],
            )
            nc.vector.tensor_copy(out_tile[:, dcol : dcol + 1], dist[:])

        # row stat columns
        nc.vector.tensor_copy(out_tile[:, 0:1], seq_f[:])
        nc.vector.tensor_copy(out_tile[:, 1:2], s_row[:])
        tmp1 = work.tile([PT, 1], F32, tag="tmp1")
        rec1 = work.tile([PT, 1], F32, tag="rec1")
        nc.vector.tensor_scalar(tmp1[:], seq_f[:], EPS, None, OP.add)
        nc.vector.reciprocal(rec1[:], tmp1[:])
        mean_row = out_tile[:, 2:3]
        nc.vector.tensor_tensor(mean_row, s_row[:], rec1[:], OP.mult)
        nc.vector.tensor_tensor(tmp1[:], s_row[:], mean_row, OP.mult)
        nc.vector.tensor_tensor(tmp1[:], ss_row[:], tmp1[:], OP.subtract)
        nc.vector.tensor_scalar(tmp1[:], tmp1[:], 0.0, None, OP.max)
        nc.vector.tensor_scalar(rec1[:], seq_f[:], 1.0, 0.0, OP.subtract, OP.max)
        nc.vector.tensor_scalar(rec1[:], rec1[:], EPS, None, OP.add)
        nc.vector.reciprocal(rec1[:], rec1[:])
        nc.vector.tensor_tensor(tmp1[:], tmp1[:], rec1[:], OP.mult)
        nc.scalar.sqrt(out_tile[:, 3:4], tmp1[:])

        nc.sync.dma_start(out_d[rows, :], out_tile[:])

    # software pipeline: emit tile t+1's scan/scatter phase before tile t's
    # accumulation phase so GPSIMD scatters overlap DVE accums
    prev = None
    for it in range(NT):
        cur = phase_a(it)
        if prev is not None:
            phase_b(prev)
        prev = cur
    phase_b(prev)


_CACHED_NC = None


def _get_nc():
    global _CACHED_NC
    if _CACHED_NC is None:
        nc = bacc.Bacc(
            "TRN2",
            target_bir_lowering=False,
            debug=False,
            num_devices=NCORES,
        )
        with ExitStack() as ctx:
            tc = ctx.enter_context(tile.TileContext(nc))
            _build_body(ctx, tc)
        nc.finalize()
        _CACHED_NC = nc
    return _CACHED_NC


def kernel(amount, mcc, tr_type, seq_lens, trace=False, **trace_kwargs):
    nc = _get_nc()
    in_maps = []
    for i in range(NCORES):
        rows = slice(i * RPC, (i + 1) * RPC)
        in_maps.append(
            {
                "amount": np.ascontiguousarray(amount[rows], dtype=np.float32),
                "mcc": np.ascontiguousarray(mcc[rows], dtype=np.int32),
                "tr_type": np.ascontiguousarray(tr_type[rows], dtype=np.int32),
                "seq_lens": np.ascontiguousarray(
                    seq_lens[rows].reshape(RPC, 1), dtype=np.int32
                ),
            }
        )
    res = run_bass_kernel_spmd(
        nc, in_maps, list(range(NCORES)), trace=trace, **trace_kwargs
    )
    out = np.concatenate([r["out"] for r in res.results], axis=0)
    if trace:
        kernel.last_result = res
    return out


# revision 25
# speedup vs baseline: 1.1925x; 1.1925x over previous
"""Trainium2 Bass kernel for nn_AggFeatureModel (segment_reduce).

Computes, per batch row b (B=4096, T=2048):
  - seq_len, sum/mean/std of amount over the full T axis
  - per-category (mcc: C=100, tr_type: C=50) count/mean/std of amount
  - distinct-category counts
Output: [B, 456].

Sharding: pure data parallel, B split across 8 NeuronCores (512 rows each).

Algorithm (per 128-row tile, per categorical stream):
 1. Partition each row's 2048 elements into 13 contiguous category-range
    groups via fused custom DVE ops (range mask + prefix-scan rank + dest,
    one op per group, chained through a fill tensor; last op emits int16).
 2. gpsimd local_scatter rearranges (cat, val) into fixed per-group windows
    of a compact grid (two scatter halves, each < 2048 dst elems).
    Phase emission is software-pipelined one tile ahead so scatters overlap
    the previous tile's accumulation.
 3. Per category, two fused scalar_tensor_tensor+accum ops over only that
    category's ~240-wide group window (instead of 2048):
      u  = sum((cat==c) * (val + 32768))  -> cnt = round(u/2^15), s = u - 2^15*cnt
      ss = sum((cat==c) * val^2)
 4. Postprocess cnt/mean/std + row stats + distinct counts.
"""

import sys

sys.path.insert(0, "/opt/trn_rl_repo")

from contextlib import ExitStack

import numpy as np

import concourse.tile as tile
from concourse import bacc, mybir, library_config
from concourse import dve_ops as DO
from concourse.dve_spec import Spec, Src0, Src1, C0, C1, C2, Zero, One, select, scan, lower
from concourse.dve_uop import DveOpSpec, AluOp
from concourse.dve_ops import has_src1
from concourse.bass_utils import run_bass_kernel_spmd

B, T = 4096, 2048
NCORES = 8
RPC = B // NCORES
C_MCC, C_TR = 100, 50
EPS = 1e-9
OUT_COLS = 456
PT = 128
NT = RPC // PT

PACK = 32768.0  # cnt/s packing offset

F32 = mybir.dt.float32
BF16 = mybir.dt.bfloat16
F16 = mybir.dt.float16
I16 = mybir.dt.int16
I32 = mybir.dt.int32
AX = mybir.AxisListType.X
OP = mybir.AluOpType
AF = mybir.ActivationFunctionType

# --- group layout -----------------------------------------------------------
# mcc: cat 1..99, 13 groups of 8 (g = (c-1)>>3); tr: cat 1..49, 13 groups of 4.
MCC_W = [240] * 12 + [110]
TR_W = [240] * 12 + [80]
MCC_NH0 = 6  # groups in scatter half 0
TR_NH0 = 6
MCC_BOUNDS = [(8 * g + 1, 8 * g + 9 if g < 12 else 16384) for g in range(13)]
TR_BOUNDS = [(4 * g + 1, 4 * g + 5 if g < 12 else 16384) for g in range(13)]


def _layout(widths, nhalf0):
    """Return (total, half0_len, half1_len, global_offsets, local_offsets)."""
    goff, loff = [], []
    g0 = 0
    for i, w in enumerate(widths[:nhalf0]):
        goff.append(g0)
        loff.append(g0)
        g0 += w
    h0 = g0
    l1 = 0
    for w in widths[nhalf0:]:
        goff.append(h0 + l1)
        loff.append(l1)
        l1 += w
    return h0 + l1, h0, l1, goff, loff


MCC_TOT, MCC_H0, MCC_H1, MCC_GOFF, MCC_LOFF = _layout(MCC_W, MCC_NH0)
TR_TOT, TR_H0, TR_H1, TR_GOFF, TR_LOFF = _layout(TR_W, TR_NH0)
assert MCC_H0 <= 2046 and MCC_H1 <= 2046 and TR_H0 <= 2046 and TR_H1 <= 2046

# --- custom DVE ops ---------------------------------------------------------
_m = (Src0 >= C0) & (Src0 < C1)
_r = scan(AluOp.ADD, _m)


def _ref_init(in0, in1, s0, s1, imm2):
    m = (in0 >= s0) & (in0 < s1)
    r = np.cumsum(m.reshape(in0.shape[0], -1), axis=-1).reshape(in0.shape)
    return np.where(m, r + imm2, -1.0).astype(np.float32)


def _ref_acc(in0, in1, s0, s1, imm2):
    m = (in0 >= s0) & (in0 < s1)
    r = np.cumsum(m.reshape(in0.shape[0], -1), axis=-1).reshape(in0.shape)
    return np.where(m, r + imm2, in1).astype(np.float32)


def _register_op(name, spec, subdim=False):
    for op in DO.OPS:
        if op.name == name:
            return op
    row = DO._CUSTOM_DVE_ROW_BASE + len(DO.OPS)
    assert row < 0x20
    shas = {}
    for ver in ("v3", "v4"):
        uops = lower(spec, ver=ver)
        s = DveOpSpec(name=name, opcode=row, uops=uops, rd1_en=has_src1(spec))
        shas[ver] = s.sha(ver)
    op = DO.DveOp(name, spec, subdim=subdim, uops_sha=shas)
    DO.OPS.append(op)
    DO._SUB_OPCODE_FOR_NAME[name] = row
    DO.CUSTOM_DVE_SPECS[name] = spec
    return op


GROUP_DEST_INIT = _register_op(
    "GROUP_DEST_INIT", Spec(body=select(_m, _r + C2, Zero - One), reference=_ref_init)
)
GROUP_DEST_ACC = _register_op(
    "GROUP_DEST_ACC", Spec(body=select(_m, _r + C2, Src1), reference=_ref_acc)
)


def _cat_stats_postproc(nc, pool, cnt, s, ss, out_tile, col0, C, epsb):
    """cnt/s/ss [128, C] f32 -> out_tile[:, col0:col0+3C]; returns distinct."""
    tmp = pool.tile([PT, C], F32, tag=f"pp_tmp_{C}")
    rec = pool.tile([PT, C], F32, tag=f"pp_rec_{C}")
    nc.vector.tensor_copy(out_tile[:, col0 : col0 + C], cnt[:])
    nc.vector.tensor_scalar(tmp[:], cnt[:], EPS, None, OP.add)
    nc.vector.reciprocal(rec[:], tmp[:])
    mean = out_tile[:, col0 + C : col0 + 2 * C]
    nc.vector.tensor_tensor(mean, s[:], rec[:], OP.mult)
    nc.vector.tensor_tensor(tmp[:], s[:], mean, OP.mult)
    nc.vector.tensor_tensor(tmp[:], ss[:], tmp[:], OP.subtract)
    nc.vector.tensor_scalar(tmp[:], tmp[:], 0.0, None, OP.max)
    nc.vector.tensor_scalar(rec[:], cnt[:], 1.0, 0.0, OP.subtract, OP.max)
    nc.vector.tensor_scalar(rec[:], rec[:], EPS, None, OP.add)
    nc.vector.reciprocal(rec[:], rec[:])
    nc.vector.tensor_tensor(tmp[:], tmp[:], rec[:], OP.mult)
    nc.scalar.sqrt(out_tile[:, col0 + 2 * C : col0 + 3 * C], tmp[:])
    dist = pool.tile([PT, 1], F32, tag=f"pp_dist_{C}")
    nc.vector.tensor_scalar(
        tmp[:], cnt[:], 0.0, 0.0, OP.is_gt, OP.add, accum_out=dist[:]
    )
    return dist


def _stream(nc, work, scat, cat_bf, val_bf, tag, widths, tot, h0len, h1len,
            goff, loff, bounds, nh0, packb=None):
    """Group-scatter one categorical stream; return (cat_s, in1f, v2_s)."""
    ngrp = len(widths)
    # dest computation: chained custom ops, one per group; two halves;
    # ping-pong between two scratch tiles (all DVE, single-buffered)
    dh = [
        work.tile([PT, T], F16, tag=f"dhA_{tag}", name=f"dhA_{tag}", bufs=1),
        work.tile([PT, T], F16, tag=f"dhB_{tag}", name=f"dhB_{tag}", bufs=1),
    ]
    idxs = []
    for half, grange in ((0, range(0, nh0)), (1, range(nh0, ngrp))):
        dprev = None
        idx = work.tile(
            [PT, T], I16, tag=f"i{half}_{tag}", name=f"i{half}_{tag}"
        )
        glist = list(grange)
        for k, g in enumerate(glist):
            op = GROUP_DEST_INIT if k == 0 else GROUP_DEST_ACC
            kw = {} if k == 0 else {"in1": dprev[:]}
            last = k == len(glist) - 1
            cur = idx if last else dh[k % 2]
            nc.vector._custom_dve(
                op, out=cur[:], in0=cat_bf[:],
                s0=float(bounds[g][0]), s1=float(bounds[g][1]),
                imm2=float(loff[g] - 1), **kw,
            )
            dprev = cur
        idxs.append(idx)
    idx0, idx1 = idxs

    cat_s = scat.tile([PT, tot], BF16, tag=f"cs_{tag}")
    val_s = scat.tile([PT, tot], BF16, tag=f"vs_{tag}")
    nc.gpsimd.local_scatter(
        cat_s[:, 0:h0len], cat_bf[:], idx0[:], channels=PT,
        num_elems=h0len, num_idxs=T,
    )
    nc.gpsimd.local_scatter(
        cat_s[:, h0len:tot], cat_bf[:], idx1[:], channels=PT,
        num_elems=h1len, num_idxs=T,
    )
    nc.gpsimd.local_scatter(
        val_s[:, 0:h0len], val_bf[:], idx0[:], channels=PT,
        num_elems=h0len, num_idxs=T,
    )
    nc.gpsimd.local_scatter(
        val_s[:, h0len:tot], val_bf[:], idx1[:], channels=PT,
        num_elems=h1len, num_idxs=T,
    )

    # payloads: in1f = val_s + PACK (f32, ACT); v2_s = val_s^2 (bf16, ACT)
    in1f = scat.tile([PT, tot], F32, tag=f"p_{tag}", bufs=1)
    nc.scalar.activation(in1f[:], val_s[:], AF.Identity, bias=packb[:])
    v2_s = scat.tile([PT, tot], BF16, tag=f"v2_{tag}", bufs=1)
    nc.scalar.activation(v2_s[:], val_s[:], AF.Square)
    return cat_s, in1f, v2_s


def _build_body(ctx, tc):
    nc = tc.nc
    amount_d = nc.dram_tensor("amount", [RPC, T], F32, kind="ExternalInput")
    mcc_d = nc.dram_tensor("mcc", [RPC, T], I32, kind="ExternalInput")
    tr_d = nc.dram_tensor("tr_type", [RPC, T], I32, kind="ExternalInput")
    seq_d = nc.dram_tensor("seq_lens", [RPC, 1], I32, kind="ExternalInput")
    out_d = nc.dram_tensor("out", [RPC, OUT_COLS], F32, kind="ExternalOutput")

    io_pool = ctx.enter_context(tc.tile_pool(name="io", bufs=1))
    work = ctx.enter_context(tc.tile_pool(name="work", bufs=2))
    scat = ctx.enter_context(tc.tile_pool(name="scat", bufs=2))
    acc_pool = ctx.enter_context(tc.tile_pool(name="acc", bufs=2))

    nc.gpsimd.load_library(library_config.local_scatter)

    packb_box = [None]
    epsb_box = [None]

    def phase_a(it):
        r0 = it * PT
        rows = slice(r0, r0 + PT)

        a = io_pool.tile([PT, T], F32, tag="a")
        nc.sync.dma_start(a[:], amount_d[rows, :])
        mcc_i = io_pool.tile([PT, T], I32, tag="mcc_i")
        nc.sync.dma_start(mcc_i[:], mcc_d[rows, :])
        tr_i = io_pool.tile([PT, T], I32, tag="tr_i")
        nc.sync.dma_start(tr_i[:], tr_d[rows, :])
        seq_i = io_pool.tile([PT, 1], I32, tag="seq_i")
        nc.sync.dma_start(seq_i[:], seq_d[rows, :])

        if packb_box[0] is None:
            packb_box[0] = work.tile([PT, 1], F32, tag="packb", name="packb", bufs=1)
            nc.vector.memset(packb_box[0][:], PACK)
            epsb_box[0] = work.tile([PT, 1], F32, tag="epsb", name="epsb", bufs=1)
            nc.vector.memset(epsb_box[0][:], EPS)
        packb = packb_box[0]
        seq_f = work.tile([PT, 1], F32, tag="seq_f")
        nc.vector.tensor_copy(seq_f[:], seq_i[:])
        mcc_bf = work.tile([PT, T], BF16, tag="mcc_bf")
        nc.scalar.copy(mcc_bf[:], mcc_i[:])
        tr_bf = work.tile([PT, T], BF16, tag="tr_bf")
        nc.scalar.copy(tr_bf[:], tr_i[:])
        val_bf = work.tile([PT, T], BF16, tag="val_bf")
        nc.scalar.copy(val_bf[:], a[:])

        # row stats on the unsorted tile
        ss_row = work.tile([PT, 1], F32, tag="ss_row")
        jact = work.tile([PT, T], BF16, tag="jact", bufs=1)
        nc.scalar.activation(jact[:], a[:], AF.Square, accum_out=ss_row[:])
        s_row = work.tile([PT, 1], F32, tag="s_row")
        nc.vector.reduce_sum(s_row[:], a[:], axis=AX)

        cat_sm, in1f_m, v2_sm = _stream(
            nc, work, scat, mcc_bf, val_bf, "m", MCC_W, MCC_TOT,
            MCC_H0, MCC_H1, MCC_GOFF, MCC_LOFF, MCC_BOUNDS, MCC_NH0, packb,
        )
        cat_st, in1f_t, v2_st = _stream(
            nc, work, scat, tr_bf, val_bf, "t", TR_W, TR_TOT,
            TR_H0, TR_H1, TR_GOFF, TR_LOFF, TR_BOUNDS, TR_NH0, packb,
        )
        return dict(
            rows=rows, seq_f=seq_f, s_row=s_row, ss_row=ss_row,
            cat_sm=cat_sm, in1f_m=in1f_m, v2_sm=v2_sm,
            cat_st=cat_st, in1f_t=in1f_t, v2_st=v2_st,
        )

    def phase_b(st):
        rows = st["rows"]
        seq_f, s_row, ss_row = st["seq_f"], st["s_row"], st["ss_row"]
        cat_sm, in1f_m, v2_sm = st["cat_sm"], st["in1f_m"], st["v2_sm"]
        cat_st, in1f_t, v2_st = st["cat_st"], st["in1f_t"], st["v2_st"]

        out_tile = acc_pool.tile([PT, OUT_COLS], F32, tag="out_tile")

        for tag, cat_s, in1f, v2_s, widths, goff, cpg, C, col0, dcol in (
            ("m", cat_sm, in1f_m, v2_sm, MCC_W, MCC_GOFF, 8, C_MCC, 4, 454),
            ("t", cat_st, in1f_t, v2_st, TR_W, TR_GOFF, 4, C_TR,
             4 + 3 * C_MCC, 455),
        ):
            u_acc = acc_pool.tile([PT, C], F32, tag=f"u_{tag}", name=f"u_{tag}")
            ss_acc = acc_pool.tile([PT, C], F32, tag=f"q_{tag}", name=f"q_{tag}")
            nc.vector.memset(u_acc[:, 0:1], 0.0)
            nc.vector.memset(ss_acc[:, 0:1], 0.0)
            sv0 = work.tile([PT, 240], BF16, tag=f"sv0_{tag}", name=f"sv0_{tag}")
            sv1 = work.tile([PT, 240], F32, tag=f"sv1_{tag}", name=f"sv1_{tag}")
            for c in range(1, C):
                g = (c - 1) // cpg
                w = widths[g]
                win = slice(goff[g], goff[g] + w)
                fc = float(c)
                # u = sum((cat==c) * (val + PACK))  [cnt+s packed]
                nc.vector.scalar_tensor_tensor(
                    sv1[:, 0:w], cat_s[:, win], fc, in1f[:, win],
                    OP.is_equal, OP.mult, accum_out=u_acc[:, c : c + 1],
                )
                # ss = sum((cat==c) * val^2)
                nc.vector.scalar_tensor_tensor(
                    sv0[:, 0:w], cat_s[:, win], fc, v2_s[:, win],
                    OP.is_equal, OP.mult, accum_out=ss_acc[:, c : c + 1],
                )

            # decode cnt/s from u = PACK*cnt + s via int convert, then a
            # +/-1 correction that works whether the convert rounds or
            # truncates
            cnt_f = acc_pool.tile([PT, C], F32, tag=f"cf_{tag}", name=f"cf_{tag}")
            cnt_i = acc_pool.tile([PT, C], I32, tag=f"ci_{tag}", name=f"ci_{tag}")
            s_dec = acc_pool.tile([PT, C], F32, tag=f"sd_{tag}", name=f"sd_{tag}")
            fix = acc_pool.tile([PT, C], F32, tag=f"fx_{tag}", name=f"fx_{tag}")
            nc.vector.tensor_scalar(cnt_f[:], u_acc[:], 1.0 / PACK, None, OP.mult)
            nc.vector.tensor_copy(cnt_i[:], cnt_f[:])
            nc.vector.tensor_copy(cnt_f[:], cnt_i[:])
            nc.vector.tensor_scalar(s_dec[:], cnt_f[:], -PACK, None, OP.mult)
            nc.vector.tensor_tensor(s_dec[:], u_acc[:], s_dec[:], OP.add)
            # if s > PACK/2: cnt += 1, s -= PACK
            nc.vector.tensor_scalar(fix[:], s_dec[:], PACK / 2, None, OP.is_gt)
            nc.vector.tensor_tensor(cnt_f[:], cnt_f[:], fix[:], OP.add)
            nc.vector.tensor_scalar(fix[:], fix[:], -PACK, None, OP.mult)
            nc.vector.tensor_tensor(s_dec[:], s_dec[:], fix[:], OP.add)
            # zero out category 0 columns
            nc.vector.memset(cnt_f[:, 0:1], 0.0)
            nc.vector.memset(s_dec[:, 0:1], 0.0)
            nc.vector.memset(ss_acc[:, 0:1], 0.0)

            dist = _cat_stats_postproc(
                nc, work, cnt_f, s_dec, ss_acc, out_tile, col0, C,
                epsb_box[0],
            )
            nc.vector.tensor_copy(out_tile[:, dcol : dcol + 1], dist[:])

        # row stat columns
        nc.vector.tensor_copy(out_tile[:, 0:1], seq_f[:])
        nc.vector.tensor_copy(out_tile[:, 1:2], s_row[:])
        tmp1 = work.tile([PT, 1], F32, tag="tmp1")
        rec1 = work.tile([PT, 1], F32, tag="rec1")
        nc.vector.tensor_scalar(tmp1[:], seq_f[:], EPS, None, OP.add)
        nc.vector.reciprocal(rec1[:], tmp1[:])
        mean_row = out_tile[:, 2:3]
        nc.vector.tensor_tensor(mean_row, s_row[:], rec1[:], OP.mult)
        nc.vector.tensor_tensor(tmp1[:], s_row[:], mean_row, OP.mult)
        nc.vector.tensor_tensor(tmp1[:], ss_row[:], tmp1[:], OP.subtract)
        nc.vector.tensor_scalar(tmp1[:], tmp1[:], 0.0, None, OP.max)
        nc.vector.tensor_scalar(rec1[:], seq_f[:], 1.0, 0.0, OP.subtract, OP.max)
        nc.vector.tensor_scalar(rec1[:], rec1[:], EPS, None, OP.add)
        nc.vector.reciprocal(rec1[:], rec1[:])
        nc.vector.tensor_tensor(tmp1[:], tmp1[:], rec1[:], OP.mult)
        nc.scalar.sqrt(out_tile[:, 3:4], tmp1[:])

        nc.sync.dma_start(out_d[rows, :], out_tile[:])

    # software pipeline: emit tile t+1's scan/scatter phase before tile t's
    # accumulation phase so GPSIMD scatters overlap DVE accums
    prev = None
    for it in range(NT):
        cur = phase_a(it)
        if prev is not None:
            phase_b(prev)
        prev = cur
    phase_b(prev)


_CACHED_NC = None


def _get_nc():
    global _CACHED_NC
    if _CACHED_NC is None:
        nc = bacc.Bacc(
            "TRN2",
            target_bir_lowering=False,
            debug=False,
            num_devices=NCORES,
        )
        with ExitStack() as ctx:
            tc = ctx.enter_context(tile.TileContext(nc))
            _build_body(ctx, tc)
        nc.finalize()
        _CACHED_NC = nc
    return _CACHED_NC


def kernel(amount, mcc, tr_type, seq_lens, trace=False, **trace_kwargs):
    nc = _get_nc()
    in_maps = []
    for i in range(NCORES):
        rows = slice(i * RPC, (i + 1) * RPC)
        in_maps.append(
            {
                "amount": np.ascontiguousarray(amount[rows], dtype=np.float32),
                "mcc": np.ascontiguousarray(mcc[rows], dtype=np.int32),
                "tr_type": np.ascontiguousarray(tr_type[rows], dtype=np.int32),
                "seq_lens": np.ascontiguousarray(
                    seq_lens[rows].reshape(RPC, 1), dtype=np.int32
                ),
            }
        )
    res = run_bass_kernel_spmd(
        nc, in_maps, list(range(NCORES)), trace=trace, **trace_kwargs
    )
    out = np.concatenate([r["out"] for r in res.results], axis=0)
    if trace:
        kernel.last_result = res
    return out


# revision 26
# speedup vs baseline: 1.3122x; 1.1003x over previous
"""Trainium2 Bass kernel for nn_AggFeatureModel (segment_reduce).

Computes, per batch row b (B=4096, T=2048):
  - seq_len, sum/mean/std of amount over the full T axis
  - per-category (mcc: C=100, tr_type: C=50) count/mean/std of amount
  - distinct-category counts
Output: [B, 456].

Sharding: pure data parallel, B split across 8 NeuronCores (512 rows each).

Algorithm (per 128-row tile, per categorical stream):
 1. Partition each row's 2048 elements into 13 contiguous category-range
    groups via fused custom DVE ops (range mask + prefix-scan rank + dest,
    one op per group, chained through a fill tensor; last op emits int16).
 2. gpsimd local_scatter rearranges (cat, val) into fixed per-group windows
    of a compact grid (two scatter halves, each < 2048 dst elems).
    Phase emission is software-pipelined one tile ahead so scatters overlap
    the previous tile's accumulation.
 3. Per category, two fused scalar_tensor_tensor+accum ops over only that
    category's ~240-wide group window (instead of 2048):
      u  = sum((cat==c) * (val + 32768))  -> cnt = round(u/2^15), s = u - 2^15*cnt
      ss = sum((cat==c) * val^2)
 4. Postprocess cnt/mean/std + row stats + distinct counts.
"""

import sys

sys.path.insert(0, "/opt/trn_rl_repo")

from contextlib import ExitStack

import numpy as np

import concourse.tile as tile
from concourse import bacc, mybir, library_config
from concourse import dve_ops as DO
from concourse.dve_spec import Spec, Src0, Src1, C0, C1, C2, Zero, One, select, scan, lower
from concourse.dve_uop import DveOpSpec, AluOp
from concourse.dve_ops import has_src1
from concourse.bass_utils import run_bass_kernel_spmd

B, T = 4096, 2048
NCORES = 8
RPC = B // NCORES
C_MCC, C_TR = 100, 50
EPS = 1e-9
OUT_COLS = 456
PT = 128
NT = RPC // PT

PACK = 128.0  # cnt/s packing offset; also ss via ACT Square(x/PACK - 1)

F32 = mybir.dt.float32
BF16 = mybir.dt.bfloat16
F16 = mybir.dt.float16
I16 = mybir.dt.int16
I32 = mybir.dt.int32
AX = mybir.AxisListType.X
OP = mybir.AluOpType
AF = mybir.ActivationFunctionType

# --- group layout -----------------------------------------------------------
# mcc: cat 1..99, 13 groups of 8 (g = (c-1)>>3); tr: cat 1..49, 13 groups of 4.
MCC_W = [240] * 12 + [110]
TR_W = [240] * 12 + [80]
MCC_NH0 = 6  # groups in scatter half 0
TR_NH0 = 6
MCC_BOUNDS = [(8 * g + 1, 8 * g + 9 if g < 12 else 16384) for g in range(13)]
TR_BOUNDS = [(4 * g + 1, 4 * g + 5 if g < 12 else 16384) for g in range(13)]


def _layout(widths, nhalf0):
    """Return (total, half0_len, half1_len, global_offsets, local_offsets)."""
    goff, loff = [], []
    g0 = 0
    for i, w in enumerate(widths[:nhalf0]):
        goff.append(g0)
        loff.append(g0)
        g0 += w
    h0 = g0
    l1 = 0
    for w in widths[nhalf0:]:
        goff.append(h0 + l1)
        loff.append(l1)
        l1 += w
    return h0 + l1, h0, l1, goff, loff


MCC_TOT, MCC_H0, MCC_H1, MCC_GOFF, MCC_LOFF = _layout(MCC_W, MCC_NH0)
TR_TOT, TR_H0, TR_H1, TR_GOFF, TR_LOFF = _layout(TR_W, TR_NH0)
assert MCC_H0 <= 2046 and MCC_H1 <= 2046 and TR_H0 <= 2046 and TR_H1 <= 2046

# --- custom DVE ops ---------------------------------------------------------
_m = (Src0 >= C0) & (Src0 < C1)
_r = scan(AluOp.ADD, _m)


def _ref_init(in0, in1, s0, s1, imm2):
    m = (in0 >= s0) & (in0 < s1)
    r = np.cumsum(m.reshape(in0.shape[0], -1), axis=-1).reshape(in0.shape)
    return np.where(m, r + imm2, -1.0).astype(np.float32)


def _ref_acc(in0, in1, s0, s1, imm2):
    m = (in0 >= s0) & (in0 < s1)
    r = np.cumsum(m.reshape(in0.shape[0], -1), axis=-1).reshape(in0.shape)
    return np.where(m, r + imm2, in1).astype(np.float32)


def _register_op(name, spec, subdim=False):
    for op in DO.OPS:
        if op.name == name:
            return op
    row = DO._CUSTOM_DVE_ROW_BASE + len(DO.OPS)
    assert row < 0x20
    shas = {}
    for ver in ("v3", "v4"):
        uops = lower(spec, ver=ver)
        s = DveOpSpec(name=name, opcode=row, uops=uops, rd1_en=has_src1(spec))
        shas[ver] = s.sha(ver)
    op = DO.DveOp(name, spec, subdim=subdim, uops_sha=shas)
    DO.OPS.append(op)
    DO._SUB_OPCODE_FOR_NAME[name] = row
    DO.CUSTOM_DVE_SPECS[name] = spec
    return op


GROUP_DEST_INIT = _register_op(
    "GROUP_DEST_INIT", Spec(body=select(_m, _r + C2, Zero - One), reference=_ref_init)
)
GROUP_DEST_ACC = _register_op(
    "GROUP_DEST_ACC", Spec(body=select(_m, _r + C2, Src1), reference=_ref_acc)
)


def _cat_stats_postproc(nc, pool, cnt, s, ss, out_tile, col0, C, epsb):
    """cnt/s/ss [128, C] f32 -> out_tile[:, col0:col0+3C]; returns distinct."""
    tmp = pool.tile([PT, C], F32, tag=f"pp_tmp_{C}")
    rec = pool.tile([PT, C], F32, tag=f"pp_rec_{C}")
    nc.vector.tensor_copy(out_tile[:, col0 : col0 + C], cnt[:])
    nc.vector.tensor_scalar(tmp[:], cnt[:], EPS, None, OP.add)
    nc.vector.reciprocal(rec[:], tmp[:])
    mean = out_tile[:, col0 + C : col0 + 2 * C]
    nc.vector.tensor_tensor(mean, s[:], rec[:], OP.mult)
    nc.vector.tensor_tensor(tmp[:], s[:], mean, OP.mult)
    nc.vector.tensor_tensor(tmp[:], ss[:], tmp[:], OP.subtract)
    nc.vector.tensor_scalar(tmp[:], tmp[:], 0.0, None, OP.max)
    nc.vector.tensor_scalar(rec[:], cnt[:], 1.0, 0.0, OP.subtract, OP.max)
    nc.vector.tensor_scalar(rec[:], rec[:], EPS, None, OP.add)
    nc.vector.reciprocal(rec[:], rec[:])
    nc.vector.tensor_tensor(tmp[:], tmp[:], rec[:], OP.mult)
    nc.scalar.sqrt(out_tile[:, col0 + 2 * C : col0 + 3 * C], tmp[:])
    dist = pool.tile([PT, 1], F32, tag=f"pp_dist_{C}")
    nc.vector.tensor_scalar(
        tmp[:], cnt[:], 0.0, 0.0, OP.is_gt, OP.add, accum_out=dist[:]
    )
    return dist


def _stream(nc, work, scat, cat_bf, val_bf, tag, widths, tot, h0len, h1len,
            goff, loff, bounds, nh0, packb=None):
    """Group-scatter one categorical stream; return (cat_s, in1f, v2_s)."""
    ngrp = len(widths)
    # dest computation: chained custom ops, one per group; two halves;
    # ping-pong between two scratch tiles (all DVE, single-buffered)
    dh = [
        work.tile([PT, T], F16, tag=f"dhA_{tag}", name=f"dhA_{tag}", bufs=1),
        work.tile([PT, T], F16, tag=f"dhB_{tag}", name=f"dhB_{tag}", bufs=1),
    ]
    idxs = []
    for half, grange in ((0, range(0, nh0)), (1, range(nh0, ngrp))):
        dprev = None
        idx = work.tile(
            [PT, T], I16, tag=f"i{half}_{tag}", name=f"i{half}_{tag}"
        )
        glist = list(grange)
        for k, g in enumerate(glist):
            op = GROUP_DEST_INIT if k == 0 else GROUP_DEST_ACC
            kw = {} if k == 0 else {"in1": dprev[:]}
            last = k == len(glist) - 1
            cur = idx if last else dh[k % 2]
            nc.vector._custom_dve(
                op, out=cur[:], in0=cat_bf[:],
                s0=float(bounds[g][0]), s1=float(bounds[g][1]),
                imm2=float(loff[g] - 1), **kw,
            )
            dprev = cur
        idxs.append(idx)
    idx0, idx1 = idxs

    cat_s = scat.tile([PT, tot], BF16, tag=f"cs_{tag}")
    val_s = scat.tile([PT, tot], BF16, tag=f"vs_{tag}")
    nc.gpsimd.local_scatter(
        cat_s[:, 0:h0len], cat_bf[:], idx0[:], channels=PT,
        num_elems=h0len, num_idxs=T,
    )
    nc.gpsimd.local_scatter(
        cat_s[:, h0len:tot], cat_bf[:], idx1[:], channels=PT,
        num_elems=h1len, num_idxs=T,
    )
    nc.gpsimd.local_scatter(
        val_s[:, 0:h0len], val_bf[:], idx0[:], channels=PT,
        num_elems=h0len, num_idxs=T,
    )
    nc.gpsimd.local_scatter(
        val_s[:, h0len:tot], val_bf[:], idx1[:], channels=PT,
        num_elems=h1len, num_idxs=T,
    )

    # payload: in1f = val_s + PACK (f32, ACT)
    in1f = scat.tile([PT, tot], F32, tag=f"p_{tag}", bufs=1)
    nc.scalar.activation(in1f[:], val_s[:], AF.Identity, bias=packb[:])
    return cat_s, in1f


def _build_body(ctx, tc):
    nc = tc.nc
    amount_d = nc.dram_tensor("amount", [RPC, T], F32, kind="ExternalInput")
    mcc_d = nc.dram_tensor("mcc", [RPC, T], I32, kind="ExternalInput")
    tr_d = nc.dram_tensor("tr_type", [RPC, T], I32, kind="ExternalInput")
    seq_d = nc.dram_tensor("seq_lens", [RPC, 1], I32, kind="ExternalInput")
    out_d = nc.dram_tensor("out", [RPC, OUT_COLS], F32, kind="ExternalOutput")

    io_pool = ctx.enter_context(tc.tile_pool(name="io", bufs=1))
    work = ctx.enter_context(tc.tile_pool(name="work", bufs=2))
    scat = ctx.enter_context(tc.tile_pool(name="scat", bufs=2))
    acc_pool = ctx.enter_context(tc.tile_pool(name="acc", bufs=2))

    nc.gpsimd.load_library(library_config.local_scatter)

    packb_box = [None]
    epsb_box = [None]
    negb_box = [None]
    sclb_box = [None]
    wvec_box = [None]

    def phase_a(it):
        r0 = it * PT
        rows = slice(r0, r0 + PT)

        a = io_pool.tile([PT, T], F32, tag="a")
        nc.sync.dma_start(a[:], amount_d[rows, :])
        mcc_i = io_pool.tile([PT, T], I32, tag="mcc_i")
        nc.sync.dma_start(mcc_i[:], mcc_d[rows, :])
        tr_i = io_pool.tile([PT, T], I32, tag="tr_i")
        nc.sync.dma_start(tr_i[:], tr_d[rows, :])
        seq_i = io_pool.tile([PT, 1], I32, tag="seq_i")
        nc.sync.dma_start(seq_i[:], seq_d[rows, :])

        if packb_box[0] is None:
            packb_box[0] = work.tile([PT, 1], F32, tag="packb", name="packb", bufs=1)
            nc.vector.memset(packb_box[0][:], PACK)
            epsb_box[0] = work.tile([PT, 1], F32, tag="epsb", name="epsb", bufs=1)
            nc.vector.memset(epsb_box[0][:], EPS)
            negb_box[0] = work.tile([PT, 1], F32, tag="negb", name="negb", bufs=1)
            nc.vector.memset(negb_box[0][:], -1.0)
            sclb_box[0] = work.tile([PT, 1], F32, tag="sclb", name="sclb", bufs=1)
            nc.vector.memset(sclb_box[0][:], 1.0 / PACK)
            wv_m = work.tile([PT, C_MCC], F32, tag="wv_m", name="wv_m", bufs=1)
            wv_t = work.tile([PT, C_TR], F32, tag="wv_t", name="wv_t", bufs=1)
            for wv, widths, cpg, C in (
                (wv_m, MCC_W, 8, C_MCC), (wv_t, TR_W, 4, C_TR)
            ):
                nc.vector.memset(wv[:, 0:1], 0.0)
                for g, w in enumerate(widths):
                    lo = 1 + g * cpg
                    hi = min(lo + cpg, C)
                    if lo < C:
                        nc.vector.memset(wv[:, lo:hi], float(w))
            wvec_box[0] = (wv_m, wv_t)
        packb = packb_box[0]
        seq_f = work.tile([PT, 1], F32, tag="seq_f")
        nc.vector.tensor_copy(seq_f[:], seq_i[:])
        mcc_bf = work.tile([PT, T], BF16, tag="mcc_bf")
        nc.scalar.copy(mcc_bf[:], mcc_i[:])
        tr_bf = work.tile([PT, T], BF16, tag="tr_bf")
        nc.scalar.copy(tr_bf[:], tr_i[:])
        val_bf = work.tile([PT, T], BF16, tag="val_bf")
        nc.scalar.copy(val_bf[:], a[:])

        # row stats on the unsorted tile
        ss_row = work.tile([PT, 1], F32, tag="ss_row")
        jact = work.tile([PT, T], BF16, tag="jact", bufs=1)
        nc.scalar.activation(jact[:], a[:], AF.Square, accum_out=ss_row[:])
        s_row = work.tile([PT, 1], F32, tag="s_row")
        nc.vector.reduce_sum(s_row[:], a[:], axis=AX)

        cat_sm, in1f_m = _stream(
            nc, work, scat, mcc_bf, val_bf, "m", MCC_W, MCC_TOT,
            MCC_H0, MCC_H1, MCC_GOFF, MCC_LOFF, MCC_BOUNDS, MCC_NH0, packb,
        )
        cat_st, in1f_t = _stream(
            nc, work, scat, tr_bf, val_bf, "t", TR_W, TR_TOT,
            TR_H0, TR_H1, TR_GOFF, TR_LOFF, TR_BOUNDS, TR_NH0, packb,
        )
        return dict(
            rows=rows, seq_f=seq_f, s_row=s_row, ss_row=ss_row,
            cat_sm=cat_sm, in1f_m=in1f_m,
            cat_st=cat_st, in1f_t=in1f_t,
        )

    def phase_b(st):
        rows = st["rows"]
        seq_f, s_row, ss_row = st["seq_f"], st["s_row"], st["ss_row"]
        cat_sm, in1f_m = st["cat_sm"], st["in1f_m"]
        cat_st, in1f_t = st["cat_st"], st["in1f_t"]

        out_tile = acc_pool.tile([PT, OUT_COLS], F32, tag="out_tile")

        for tag, cat_s, in1f, wvec, widths, goff, cpg, C, col0, dcol in (
            ("m", cat_sm, in1f_m, wvec_box[0][0], MCC_W, MCC_GOFF, 8, C_MCC,
             4, 454),
            ("t", cat_st, in1f_t, wvec_box[0][1], TR_W, TR_GOFF, 4, C_TR,
             4 + 3 * C_MCC, 455),
        ):
            u_acc = acc_pool.tile([PT, C], F32, tag=f"u_{tag}", name=f"u_{tag}")
            ss_acc = acc_pool.tile([PT, C], F32, tag=f"q_{tag}", name=f"q_{tag}")
            nc.vector.memset(u_acc[:, 0:1], 0.0)
            nc.vector.memset(ss_acc[:, 0:1], 0.0)
            sv1s = [
                work.tile([PT, 240], F32, tag=f"sv1_{tag}{i}",
                          name=f"sv1_{tag}{i}")
                for i in range(3)
            ]
            jsqs = [
                work.tile([PT, 240], BF16, tag=f"jsq_{tag}{i}",
                          name=f"jsq_{tag}{i}")
                for i in range(3)
            ]
            for c in range(1, C):
                g = (c - 1) // cpg
                w = widths[g]
                win = slice(goff[g], goff[g] + w)
                fc = float(c)
                sv1 = sv1s[c % 3]
                jsq = jsqs[c % 3]
                # u = sum((cat==c) * (val + PACK))  [cnt+s packed]
                nc.vector.scalar_tensor_tensor(
                    sv1[:, 0:w], cat_s[:, win], fc, in1f[:, win],
                    OP.is_equal, OP.mult, accum_out=u_acc[:, c : c + 1],
                )
                # ss via ACT: Square(sv1/PACK - 1); masked -> (val/PACK)^2,
                # empty -> 1.  ss = (acc - W + cnt) * PACK^2
                nc.scalar.activation(
                    jsq[:, 0:w], sv1[:, 0:w], AF.Square,
                    bias=negb_box[0][:], scale=sclb_box[0][:],
                    accum_out=ss_acc[:, c : c + 1],
                )

            # decode cnt/s from u = PACK*cnt + s via int convert, then a
            # +/-1 correction that works whether the convert rounds or
            # truncates
            cnt_f = acc_pool.tile([PT, C], F32, tag=f"cf_{tag}", name=f"cf_{tag}")
            cnt_i = acc_pool.tile([PT, C], I32, tag=f"ci_{tag}", name=f"ci_{tag}")
            s_dec = acc_pool.tile([PT, C], F32, tag=f"sd_{tag}", name=f"sd_{tag}")
            fix = acc_pool.tile([PT, C], F32, tag=f"fx_{tag}", name=f"fx_{tag}")
            nc.vector.tensor_scalar(cnt_f[:], u_acc[:], 1.0 / PACK, None, OP.mult)
            nc.vector.tensor_copy(cnt_i[:], cnt_f[:])
            nc.vector.tensor_copy(cnt_f[:], cnt_i[:])
            nc.vector.tensor_scalar(s_dec[:], cnt_f[:], -PACK, None, OP.mult)
            nc.vector.tensor_tensor(s_dec[:], u_acc[:], s_dec[:], OP.add)
            # if s > PACK/2: cnt += 1, s -= PACK
            nc.vector.tensor_scalar(fix[:], s_dec[:], PACK / 2, None, OP.is_gt)
            nc.vector.tensor_tensor(cnt_f[:], cnt_f[:], fix[:], OP.add)
            nc.vector.tensor_scalar(fix[:], fix[:], -PACK, None, OP.mult)
            nc.vector.tensor_tensor(s_dec[:], s_dec[:], fix[:], OP.add)
            # decode ss = (raw - W + cnt) * PACK^2
            nc.vector.tensor_tensor(ss_acc[:], ss_acc[:], wvec[:], OP.subtract)
            nc.vector.tensor_tensor(ss_acc[:], ss_acc[:], cnt_f[:], OP.add)
            nc.vector.tensor_scalar(
                ss_acc[:], ss_acc[:], PACK * PACK, None, OP.mult
            )
            # zero out category 0 columns
            nc.vector.memset(cnt_f[:, 0:1], 0.0)
            nc.vector.memset(s_dec[:, 0:1], 0.0)
            nc.vector.memset(ss_acc[:, 0:1], 0.0)

            dist = _cat_stats_postproc(
                nc, work, cnt_f, s_dec, ss_acc, out_tile, col0, C,
                epsb_box[0],
            )
            nc.vector.tensor_copy(out_tile[:, dcol : dcol + 1], dist[:])

        # row stat columns
        nc.vector.tensor_copy(out_tile[:, 0:1], seq_f[:])
        nc.vector.tensor_copy(out_tile[:, 1:2], s_row[:])
        tmp1 = work.tile([PT, 1], F32, tag="tmp1")
        rec1 = work.tile([PT, 1], F32, tag="rec1")
        nc.vector.tensor_scalar(tmp1[:], seq_f[:], EPS, None, OP.add)
        nc.vector.reciprocal(rec1[:], tmp1[:])
        mean_row = out_tile[:, 2:3]
        nc.vector.tensor_tensor(mean_row, s_row[:], rec1[:], OP.mult)
        nc.vector.tensor_tensor(tmp1[:], s_row[:], mean_row, OP.mult)
        nc.vector.tensor_tensor(tmp1[:], ss_row[:], tmp1[:], OP.subtract)
        nc.vector.tensor_scalar(tmp1[:], tmp1[:], 0.0, None, OP.max)
        nc.vector.tensor_scalar(rec1[:], seq_f[:], 1.0, 0.0, OP.subtract, OP.max)
        nc.vector.tensor_scalar(rec1[:], rec1[:], EPS, None, OP.add)
        nc.vector.reciprocal(rec1[:], rec1[:])
        nc.vector.tensor_tensor(tmp1[:], tmp1[:], rec1[:], OP.mult)
        nc.scalar.sqrt(out_tile[:, 3:4], tmp1[:])

        nc.sync.dma_start(out_d[rows, :], out_tile[:])

    # software pipeline: emit tile t+1's scan/scatter phase before tile t's
    # accumulation phase so GPSIMD scatters overlap DVE accums
    prev = None
    for it in range(NT):
        cur = phase_a(it)
        if prev is not None:
            phase_b(prev)
        prev = cur
    phase_b(prev)


_CACHED_NC = None


def _get_nc():
    global _CACHED_NC
    if _CACHED_NC is None:
        nc = bacc.Bacc(
            "TRN2",
            target_bir_lowering=False,
            debug=False,
            num_devices=NCORES,
        )
        with ExitStack() as ctx:
            tc = ctx.enter_context(tile.TileContext(nc))
            _build_body(ctx, tc)
        nc.finalize()
        _CACHED_NC = nc
    return _CACHED_NC


def kernel(amount, mcc, tr_type, seq_lens, trace=False, **trace_kwargs):
    nc = _get_nc()
    in_maps = []
    for i in range(NCORES):
        rows = slice(i * RPC, (i + 1) * RPC)
        in_maps.append(
            {
                "amount": np.ascontiguousarray(amount[rows], dtype=np.float32),
                "mcc": np.ascontiguousarray(mcc[rows], dtype=np.int32),
                "tr_type": np.ascontiguousarray(tr_type[rows], dtype=np.int32),
                "seq_lens": np.ascontiguousarray(
                    seq_lens[rows].reshape(RPC, 1), dtype=np.int32
                ),
            }
        )
    res = run_bass_kernel_spmd(
        nc, in_maps, list(range(NCORES)), trace=trace, **trace_kwargs
    )
    out = np.concatenate([r["out"] for r in res.results], axis=0)
    if trace:
        kernel.last_result = res
    return out


# revision 29
# speedup vs baseline: 1.3439x; 1.0242x over previous
"""Trainium2 Bass kernel for nn_AggFeatureModel (segment_reduce).

Computes, per batch row b (B=4096, T=2048):
  - seq_len, sum/mean/std of amount over the full T axis
  - per-category (mcc: C=100, tr_type: C=50) count/mean/std of amount
  - distinct-category counts
Output: [B, 456].

Sharding: pure data parallel, B split across 8 NeuronCores (512 rows each).

Algorithm (per 128-row tile, per categorical stream):
 1. Partition each row's 2048 elements into 13 contiguous category-range
    groups via fused custom DVE ops (range mask + prefix-scan rank + dest,
    one op per group, chained through a fill tensor; last op emits int16).
 2. gpsimd local_scatter rearranges (cat, val) into fixed per-group windows
    of a compact grid (two scatter halves, each < 2048 dst elems).
    Phase emission is software-pipelined one tile ahead so scatters overlap
    the previous tile's accumulation.
 3. Per category, ONE fused scalar_tensor_tensor+accum op on DVE over only
    that category's ~240-wide group window (instead of 2048):
      u = sum((cat==c) * (val + 128))  -> cnt = round(u/128), s = u - 128*cnt
    and ss on the Activation engine via poison-compensated Square of the
    DVE op's output sv1 = (cat==c)*(val+128):
      ACTsum = sum(Square(sv1/128 - 1))  -> ss = (ACTsum - W + cnt) * 128^2
 4. Postprocess cnt/mean/std + row stats + distinct counts.
"""

import sys

sys.path.insert(0, "/opt/trn_rl_repo")

from contextlib import ExitStack

import numpy as np

import concourse.tile as tile
from concourse import bacc, mybir, library_config
from concourse import dve_ops as DO
from concourse.dve_spec import Spec, Src0, Src1, C0, C1, C2, Zero, One, select, scan, lower
from concourse.dve_uop import DveOpSpec, AluOp
from concourse.dve_ops import has_src1
from concourse.bass_utils import run_bass_kernel_spmd

B, T = 4096, 2048
NCORES = 8
RPC = B // NCORES
C_MCC, C_TR = 100, 50
EPS = 1e-9
OUT_COLS = 456
PT = 128
NT = RPC // PT

PACK = 128.0  # cnt/s packing offset; also ss via ACT Square(x/PACK - 1)

F32 = mybir.dt.float32
BF16 = mybir.dt.bfloat16
F16 = mybir.dt.float16
I16 = mybir.dt.int16
I32 = mybir.dt.int32
AX = mybir.AxisListType.X
OP = mybir.AluOpType
AF = mybir.ActivationFunctionType

# --- group layout -----------------------------------------------------------
# mcc: cat 1..99, 13 groups of 8 (g = (c-1)>>3); tr: cat 1..49, 13 groups of 4.
MCC_W = [240] * 12 + [110]
TR_W = [240] * 12 + [80]
MCC_NH0 = 6  # groups in scatter half 0
TR_NH0 = 6
MCC_BOUNDS = [(8 * g + 1, 8 * g + 9 if g < 12 else 16384) for g in range(13)]
TR_BOUNDS = [(4 * g + 1, 4 * g + 5 if g < 12 else 16384) for g in range(13)]


def _layout(widths, nhalf0):
    """Return (total, half0_len, half1_len, global_offsets, local_offsets)."""
    goff, loff = [], []
    g0 = 0
    for i, w in enumerate(widths[:nhalf0]):
        goff.append(g0)
        loff.append(g0)
        g0 += w
    h0 = g0
    l1 = 0
    for w in widths[nhalf0:]:
        goff.append(h0 + l1)
        loff.append(l1)
        l1 += w
    return h0 + l1, h0, l1, goff, loff


MCC_TOT, MCC_H0, MCC_H1, MCC_GOFF, MCC_LOFF = _layout(MCC_W, MCC_NH0)
TR_TOT, TR_H0, TR_H1, TR_GOFF, TR_LOFF = _layout(TR_W, TR_NH0)
assert MCC_H0 <= 2046 and MCC_H1 <= 2046 and TR_H0 <= 2046 and TR_H1 <= 2046

# --- custom DVE ops ---------------------------------------------------------
_m = (Src0 >= C0) & (Src0 < C1)
_r = scan(AluOp.ADD, _m)


def _ref_init(in0, in1, s0, s1, imm2):
    m = (in0 >= s0) & (in0 < s1)
    r = np.cumsum(m.reshape(in0.shape[0], -1), axis=-1).reshape(in0.shape)
    return np.where(m, r + imm2, -1.0).astype(np.float32)


def _ref_acc(in0, in1, s0, s1, imm2):
    m = (in0 >= s0) & (in0 < s1)
    r = np.cumsum(m.reshape(in0.shape[0], -1), axis=-1).reshape(in0.shape)
    return np.where(m, r + imm2, in1).astype(np.float32)


def _register_op(name, spec, subdim=False):
    for op in DO.OPS:
        if op.name == name:
            return op
    row = DO._CUSTOM_DVE_ROW_BASE + len(DO.OPS)
    assert row < 0x20
    shas = {}
    for ver in ("v3", "v4"):
        uops = lower(spec, ver=ver)
        s = DveOpSpec(name=name, opcode=row, uops=uops, rd1_en=has_src1(spec))
        shas[ver] = s.sha(ver)
    op = DO.DveOp(name, spec, subdim=subdim, uops_sha=shas)
    DO.OPS.append(op)
    DO._SUB_OPCODE_FOR_NAME[name] = row
    DO.CUSTOM_DVE_SPECS[name] = spec
    return op


GROUP_DEST_INIT = _register_op(
    "GROUP_DEST_INIT", Spec(body=select(_m, _r + C2, Zero - One), reference=_ref_init)
)
GROUP_DEST_ACC = _register_op(
    "GROUP_DEST_ACC", Spec(body=select(_m, _r + C2, Src1), reference=_ref_acc)
)


def _cat_stats_postproc(nc, pool, cnt, s, ss, out_tile, col0, C, epsb):
    """cnt/s/ss [128, C] f32 -> out_tile[:, col0:col0+3C]; returns distinct."""
    tmp = pool.tile([PT, C], F32, tag=f"pp_tmp_{C}")
    rec = pool.tile([PT, C], F32, tag=f"pp_rec_{C}")
    nc.vector.tensor_copy(out_tile[:, col0 : col0 + C], cnt[:])
    nc.vector.tensor_scalar(tmp[:], cnt[:], EPS, None, OP.add)
    nc.vector.reciprocal(rec[:], tmp[:])
    mean = out_tile[:, col0 + C : col0 + 2 * C]
    nc.vector.tensor_tensor(mean, s[:], rec[:], OP.mult)
    nc.vector.tensor_tensor(tmp[:], s[:], mean, OP.mult)
    nc.vector.tensor_tensor(tmp[:], ss[:], tmp[:], OP.subtract)
    nc.vector.tensor_scalar(tmp[:], tmp[:], 0.0, None, OP.max)
    nc.vector.tensor_scalar(rec[:], cnt[:], 1.0, 0.0, OP.subtract, OP.max)
    nc.vector.tensor_scalar(rec[:], rec[:], EPS, None, OP.add)
    nc.vector.reciprocal(rec[:], rec[:])
    nc.vector.tensor_tensor(tmp[:], tmp[:], rec[:], OP.mult)
    nc.scalar.sqrt(out_tile[:, col0 + 2 * C : col0 + 3 * C], tmp[:])
    dist = pool.tile([PT, 1], F32, tag=f"pp_dist_{C}")
    nc.vector.tensor_scalar(
        tmp[:], cnt[:], 0.0, 0.0, OP.is_gt, OP.add, accum_out=dist[:]
    )
    return dist


def _stream(nc, work, scat, cat_bf, val_bf, tag, widths, tot, h0len, h1len,
            goff, loff, bounds, nh0, packb=None, p_bufs=1):
    """Group-scatter one categorical stream; return (cat_s, in1f)."""
    ngrp = len(widths)
    # dest computation: chained custom ops, one per group; two halves;
    # ping-pong between two scratch tiles (all DVE, single-buffered)
    dh = [
        work.tile([PT, T], F16, tag="dhA", name="dhA", bufs=1),
        work.tile([PT, T], F16, tag="dhB", name="dhB", bufs=1),
    ]
    idxs = []
    for half, grange in ((0, range(0, nh0)), (1, range(nh0, ngrp))):
        dprev = None
        idx = work.tile(
            [PT, T], I16, tag=f"i{half}_{tag}", name=f"i{half}_{tag}"
        )
        glist = list(grange)
        for k, g in enumerate(glist):
            op = GROUP_DEST_INIT if k == 0 else GROUP_DEST_ACC
            kw = {} if k == 0 else {"in1": dprev[:]}
            last = k == len(glist) - 1
            cur = idx if last else dh[k % 2]
            nc.vector._custom_dve(
                op, out=cur[:], in0=cat_bf[:],
                s0=float(bounds[g][0]), s1=float(bounds[g][1]),
                imm2=float(loff[g] - 1), **kw,
            )
            dprev = cur
        idxs.append(idx)
    idx0, idx1 = idxs

    cat_s = scat.tile([PT, tot], BF16, tag=f"cs_{tag}")
    val_s = scat.tile([PT, tot], BF16, tag=f"vs_{tag}")
    nc.gpsimd.local_scatter(
        cat_s[:, 0:h0len], cat_bf[:], idx0[:], channels=PT,
        num_elems=h0len, num_idxs=T,
    )
    nc.gpsimd.local_scatter(
        cat_s[:, h0len:tot], cat_bf[:], idx1[:], channels=PT,
        num_elems=h1len, num_idxs=T,
    )
    nc.gpsimd.local_scatter(
        val_s[:, 0:h0len], val_bf[:], idx0[:], channels=PT,
        num_elems=h0len, num_idxs=T,
    )
    nc.gpsimd.local_scatter(
        val_s[:, h0len:tot], val_bf[:], idx1[:], channels=PT,
        num_elems=h1len, num_idxs=T,
    )

    # payload: in1f = val_s + PACK (f32, ACT)
    in1f = scat.tile([PT, tot], F32, tag=f"p_{tag}", bufs=p_bufs)
    nc.scalar.activation(in1f[:], val_s[:], AF.Identity, bias=packb[:])
    return cat_s, in1f


def _build_body(ctx, tc):
    nc = tc.nc
    amount_d = nc.dram_tensor("amount", [RPC, T], F32, kind="ExternalInput")
    mcc_d = nc.dram_tensor("mcc", [RPC, T], I32, kind="ExternalInput")
    tr_d = nc.dram_tensor("tr_type", [RPC, T], I32, kind="ExternalInput")
    seq_d = nc.dram_tensor("seq_lens", [RPC, 1], I32, kind="ExternalInput")
    out_d = nc.dram_tensor("out", [RPC, OUT_COLS], F32, kind="ExternalOutput")

    io_pool = ctx.enter_context(tc.tile_pool(name="io", bufs=1))
    work = ctx.enter_context(tc.tile_pool(name="work", bufs=2))
    scat = ctx.enter_context(tc.tile_pool(name="scat", bufs=2))
    acc_pool = ctx.enter_context(tc.tile_pool(name="acc", bufs=2))

    nc.gpsimd.load_library(library_config.local_scatter)

    packb_box = [None]
    epsb_box = [None]
    negb_box = [None]
    sclb_box = [None]
    wvec_box = [None]

    def phase_a(it):
        r0 = it * PT
        rows = slice(r0, r0 + PT)

        a = io_pool.tile([PT, T], F32, tag="a")
        nc.sync.dma_start(a[:], amount_d[rows, :])
        mcc_i = io_pool.tile([PT, T], I32, tag="mcc_i")
        nc.sync.dma_start(mcc_i[:], mcc_d[rows, :])
        tr_i = io_pool.tile([PT, T], I32, tag="tr_i")
        nc.sync.dma_start(tr_i[:], tr_d[rows, :])
        seq_i = io_pool.tile([PT, 1], I32, tag="seq_i")
        nc.sync.dma_start(seq_i[:], seq_d[rows, :])

        if packb_box[0] is None:
            packb_box[0] = work.tile([PT, 1], F32, tag="packb", name="packb", bufs=1)
            nc.vector.memset(packb_box[0][:], PACK)
            epsb_box[0] = work.tile([PT, 1], F32, tag="epsb", name="epsb", bufs=1)
            nc.vector.memset(epsb_box[0][:], EPS)
            negb_box[0] = work.tile([PT, 1], F32, tag="negb", name="negb", bufs=1)
            nc.vector.memset(negb_box[0][:], -1.0)
            sclb_box[0] = work.tile([PT, 1], F32, tag="sclb", name="sclb", bufs=1)
            nc.vector.memset(sclb_box[0][:], 1.0 / PACK)
            wv_m = work.tile([PT, C_MCC], F32, tag="wv_m", name="wv_m", bufs=1)
            wv_t = work.tile([PT, C_TR], F32, tag="wv_t", name="wv_t", bufs=1)
            for wv, widths, cpg, C in (
                (wv_m, MCC_W, 8, C_MCC), (wv_t, TR_W, 4, C_TR)
            ):
                nc.vector.memset(wv[:, 0:1], 0.0)
                for g, w in enumerate(widths):
                    lo = 1 + g * cpg
                    hi = min(lo + cpg, C)
                    if lo < C:
                        nc.vector.memset(wv[:, lo:hi], float(w))
            wvec_box[0] = (wv_m, wv_t)
        packb = packb_box[0]
        seq_f = work.tile([PT, 1], F32, tag="seq_f")
        nc.vector.tensor_copy(seq_f[:], seq_i[:])
        mcc_bf = work.tile([PT, T], BF16, tag="mcc_bf")
        nc.scalar.copy(mcc_bf[:], mcc_i[:])
        tr_bf = work.tile([PT, T], BF16, tag="tr_bf")
        nc.scalar.copy(tr_bf[:], tr_i[:])
        val_bf = work.tile([PT, T], BF16, tag="val_bf")
        nc.scalar.copy(val_bf[:], a[:])

        # row stats on the unsorted tile
        ss_row = work.tile([PT, 1], F32, tag="ss_row")
        jact = work.tile([PT, T], BF16, tag="jact", bufs=1)
        nc.scalar.activation(jact[:], a[:], AF.Square, accum_out=ss_row[:])
        s_row = work.tile([PT, 1], F32, tag="s_row")
        nc.vector.reduce_sum(s_row[:], a[:], axis=AX)

        cat_sm, in1f_m = _stream(
            nc, work, scat, mcc_bf, val_bf, "m", MCC_W, MCC_TOT,
            MCC_H0, MCC_H1, MCC_GOFF, MCC_LOFF, MCC_BOUNDS, MCC_NH0, packb,
        )
        cat_st, in1f_t = _stream(
            nc, work, scat, tr_bf, val_bf, "t", TR_W, TR_TOT,
            TR_H0, TR_H1, TR_GOFF, TR_LOFF, TR_BOUNDS, TR_NH0, packb,
            p_bufs=2,
        )
        return dict(
            rows=rows, seq_f=seq_f, s_row=s_row, ss_row=ss_row,
            cat_sm=cat_sm, in1f_m=in1f_m,
            cat_st=cat_st, in1f_t=in1f_t,
        )

    def phase_b(st):
        rows = st["rows"]
        seq_f, s_row, ss_row = st["seq_f"], st["s_row"], st["ss_row"]
        cat_sm, in1f_m = st["cat_sm"], st["in1f_m"]
        cat_st, in1f_t = st["cat_st"], st["in1f_t"]

        st["out_tile"] = acc_pool.tile(
            [PT, OUT_COLS], F32, tag="out_tile", name="out_tile"
        )
        _accum_stream(st, "m")

    def _accum_stream(st, which):
        if which == "m":
            tag, cat_s, in1f, widths, goff, cpg, C = (
                "m", st["cat_sm"], st["in1f_m"], MCC_W, MCC_GOFF, 8, C_MCC)
        else:
            tag, cat_s, in1f, widths, goff, cpg, C = (
                "t", st["cat_st"], st["in1f_t"], TR_W, TR_GOFF, 4, C_TR)
        if True:
            u_acc = acc_pool.tile(
                [PT, C], F32, tag=f"u_{tag}", name=f"u_{tag}"
            )
            ss_acc = acc_pool.tile(
                [PT, C], F32, tag=f"q_{tag}", name=f"q_{tag}"
            )
            nc.vector.memset(u_acc[:, 0:1], 0.0)
            nc.vector.memset(ss_acc[:, 0:1], 0.0)
            sv1s = [
                work.tile([PT, 240], F32, tag=f"sv1_{tag}{i}",
                          name=f"sv1_{tag}{i}")
                for i in range(3)
            ]
            jsqs = [
                work.tile([PT, 240], BF16, tag=f"jsq_{tag}{i}",
                          name=f"jsq_{tag}{i}")
                for i in range(3)
            ]
            for c in range(1, C):
                g = (c - 1) // cpg
                w = widths[g]
                win = slice(goff[g], goff[g] + w)
                fc = float(c)
                sv1 = sv1s[c % 3]
                jsq = jsqs[c % 3]
                # u = sum((cat==c) * (val + PACK))  [cnt+s packed]
                nc.vector.scalar_tensor_tensor(
                    sv1[:, 0:w], cat_s[:, win], fc, in1f[:, win],
                    OP.is_equal, OP.mult, accum_out=u_acc[:, c : c + 1],
                )
                # ss via ACT: Square(sv1/PACK - 1); masked -> (val/PACK)^2,
                # empty -> 1.  ss = (acc - W + cnt) * PACK^2
                nc.scalar.activation(
                    jsq[:, 0:w], sv1[:, 0:w], AF.Square,
                    bias=negb_box[0][:], scale=sclb_box[0][:],
                    accum_out=ss_acc[:, c : c + 1],
                )

            st[f"u_{tag}"] = u_acc
            st[f"ss_{tag}"] = ss_acc

    def _decode_stream(st, which):
        if which == "m":
            tag, wvec, widths, cpg, C, col0, dcol = (
                "m", wvec_box[0][0], MCC_W, 8, C_MCC, 4, 454)
        else:
            tag, wvec, widths, cpg, C, col0, dcol = (
                "t", wvec_box[0][1], TR_W, 4, C_TR, 4 + 3 * C_MCC, 455)
        u_acc = st[f"u_{tag}"]
        ss_acc = st[f"ss_{tag}"]
        out_tile = st["out_tile"]
        if True:
            # decode cnt/s from u = PACK*cnt + s via int convert, then a
            # +/-1 correction that works whether the convert rounds or
            # truncates
            cnt_f = acc_pool.tile([PT, C], F32, tag=f"cf_{tag}", name=f"cf_{tag}")
            cnt_i = acc_pool.tile([PT, C], I32, tag=f"ci_{tag}", name=f"ci_{tag}")
            s_dec = acc_pool.tile([PT, C], F32, tag=f"sd_{tag}", name=f"sd_{tag}")
            fix = acc_pool.tile([PT, C], F32, tag=f"fx_{tag}", name=f"fx_{tag}")
            nc.vector.tensor_scalar(cnt_f[:], u_acc[:], 1.0 / PACK, None, OP.mult)
            nc.vector.tensor_copy(cnt_i[:], cnt_f[:])
            nc.vector.tensor_copy(cnt_f[:], cnt_i[:])
            nc.vector.tensor_scalar(s_dec[:], cnt_f[:], -PACK, None, OP.mult)
            nc.vector.tensor_tensor(s_dec[:], u_acc[:], s_dec[:], OP.add)
            # if s > PACK/2: cnt += 1, s -= PACK
            nc.vector.tensor_scalar(fix[:], s_dec[:], PACK / 2, None, OP.is_gt)
            nc.vector.tensor_tensor(cnt_f[:], cnt_f[:], fix[:], OP.add)
            nc.vector.tensor_scalar(fix[:], fix[:], -PACK, None, OP.mult)
            nc.vector.tensor_tensor(s_dec[:], s_dec[:], fix[:], OP.add)
            # decode ss = (raw - W + cnt) * PACK^2
            nc.vector.tensor_tensor(ss_acc[:], ss_acc[:], wvec[:], OP.subtract)
            nc.vector.tensor_tensor(ss_acc[:], ss_acc[:], cnt_f[:], OP.add)
            nc.vector.tensor_scalar(
                ss_acc[:], ss_acc[:], PACK * PACK, None, OP.mult
            )
            # zero out category 0 columns
            nc.vector.memset(cnt_f[:, 0:1], 0.0)
            nc.vector.memset(s_dec[:, 0:1], 0.0)
            nc.vector.memset(ss_acc[:, 0:1], 0.0)

            dist = _cat_stats_postproc(
                nc, work, cnt_f, s_dec, ss_acc, out_tile, col0, C,
                epsb_box[0],
            )
            nc.vector.tensor_copy(out_tile[:, dcol : dcol + 1], dist[:])

    def phase_b2(st):
        _accum_stream(st, "t")
        _decode_stream(st, "m")
        _decode_stream(st, "t")
        seq_f, s_row, ss_row = st["seq_f"], st["s_row"], st["ss_row"]
        rows = st["rows"]
        out_tile = st["out_tile"]
        # row stat columns
        nc.vector.tensor_copy(out_tile[:, 0:1], seq_f[:])
        nc.vector.tensor_copy(out_tile[:, 1:2], s_row[:])
        tmp1 = work.tile([PT, 1], F32, tag="tmp1")
        rec1 = work.tile([PT, 1], F32, tag="rec1")
        nc.vector.tensor_scalar(tmp1[:], seq_f[:], EPS, None, OP.add)
        nc.vector.reciprocal(rec1[:], tmp1[:])
        mean_row = out_tile[:, 2:3]
        nc.vector.tensor_tensor(mean_row, s_row[:], rec1[:], OP.mult)
        nc.vector.tensor_tensor(tmp1[:], s_row[:], mean_row, OP.mult)
        nc.vector.tensor_tensor(tmp1[:], ss_row[:], tmp1[:], OP.subtract)
        nc.vector.tensor_scalar(tmp1[:], tmp1[:], 0.0, None, OP.max)
        nc.vector.tensor_scalar(rec1[:], seq_f[:], 1.0, 0.0, OP.subtract, OP.max)
        nc.vector.tensor_scalar(rec1[:], rec1[:], EPS, None, OP.add)
        nc.vector.reciprocal(rec1[:], rec1[:])
        nc.vector.tensor_tensor(tmp1[:], tmp1[:], rec1[:], OP.mult)
        nc.scalar.sqrt(out_tile[:, 3:4], tmp1[:])

        nc.sync.dma_start(out_d[rows, :], out_tile[:])

    # software pipeline: emit tile t+1's scan/scatter phase BETWEEN tile t's
    # two stream loops so custom-op work fills the ACT-paced stall windows
    prev = None
    for it in range(NT):
        if prev is not None:
            phase_b(prev)      # mcc cat loop of tile t-1
            cur = phase_a(it)  # scans/scatters of tile t fill ACT lag
            phase_b2(prev)     # tr loop + decode + out of tile t-1
        else:
            cur = phase_a(it)
        prev = cur
    phase_b(prev)
    phase_b2(prev)


_CACHED_NC = None


def _get_nc():
    global _CACHED_NC
    if _CACHED_NC is None:
        nc = bacc.Bacc(
            "TRN2",
            target_bir_lowering=False,
            debug=False,
            num_devices=NCORES,
        )
        with ExitStack() as ctx:
            tc = ctx.enter_context(tile.TileContext(nc))
            _build_body(ctx, tc)
        nc.finalize()
        _CACHED_NC = nc
    return _CACHED_NC


def kernel(amount, mcc, tr_type, seq_lens, trace=False, **trace_kwargs):
    nc = _get_nc()
    in_maps = []
    for i in range(NCORES):
        rows = slice(i * RPC, (i + 1) * RPC)
        in_maps.append(
            {
                "amount": np.ascontiguousarray(amount[rows], dtype=np.float32),
                "mcc": np.ascontiguousarray(mcc[rows], dtype=np.int32),
                "tr_type": np.ascontiguousarray(tr_type[rows], dtype=np.int32),
                "seq_lens": np.ascontiguousarray(
                    seq_lens[rows].reshape(RPC, 1), dtype=np.int32
                ),
            }
        )
    res = run_bass_kernel_spmd(
        nc, in_maps, list(range(NCORES)), trace=trace, **trace_kwargs
    )
    out = np.concatenate([r["out"] for r in res.results], axis=0)
    if trace:
        kernel.last_result = res
    return out
